# revision 4
# baseline (speedup 1.0000x reference)
"""Trainium2 Bass kernel for nn_ClosedArap (ARAP rhs, GNN message passing), v5.

rhs_i = sum_k w_ik * 0.5 * (R_i + R_j) @ (p_i - p_j),  j = nbr[i, k]
      = R_i @ (sum_k w'_ik d_ik) + sum_k R_j @ (w'_ik d_ik),   w' = w/2

The axon link to the device runs at ~35 MB/s serialized, so a device
invocation's cost is dominated by uploaded bytes.  This version ships:
  - per-vertex table rows packed to 20B: p as 3x f16 (6B) + R as 9x
    12-bit fixed point (14B, scale 346.9, offset 2048); only a 1/8 shard
    is uploaded per core and an on-device AllGather replicates the full
    table into each core's DRAM,
  - neighbor ids packed to 20 bits (2.5B/edge: two byte planes + a
    nibble plane),
  - weights quantized to u8 (w' = u8/510, descaled at load),
  - everything fused into one u8 blob per core (one link transfer),
  - local rows streamed from the core's own shard (no upload),
  - rhs downloaded as f16.
The 12-bit fields are decoded on DVE with shift/and/or into f16 (offset
-2048 applied in f16; values stay exact integers <= 2048).  rhs is linear
in R, so the 1/346.9 scale folds into the final output copy.  Total rel
err vs the f32 reference ~1.7e-3 (threshold 2e-2).

Gathers are one-offset-per-partition indirect DMAs (the multi-offset
"vector dynamic offset" DMA form generates descriptors for only one
partition in this stack), rotated over 4 SWDGE queues so the per-queue
16-bit DMA-semaphore fields stay under 65535 in a single-NEFF invocation.

Slot map: core c owns vertex slots [c*SLOTS, (c+1)*SLOTS), slot (p, t)
of core c holds vertex c*SLOTS + p*NT + t; host staging is pure reshape
of [NPAD]-padded arrays.
"""
import numpy as np

from concourse import bass, bacc, mybir, tile

N_FULL = 1_000_000
K = 8
NCORES = 8
DB = 20           # packed row bytes: p 3xf16 (6) + R 9x12bit (14)
GRP = 16          # vertex tiles (of 128) per pipeline group
NT = 992          # vertex tiles per core; 128*992 = 126976 slots
SLOTS = 128 * NT
NPAD = NCORES * SLOTS          # 1015808 padded table rows (< 2^20)
NQ = 4            # SWDGE queues to rotate gathers over
SW = 510.0        # weight u8 scale (w' = w/2 = u8/510)
SC = 346.9        # rotation 12-bit scale (clips beyond +-5.9 sigma)
OPB = 5 * K // 2  # offset bytes per vertex slot (2.5 per edge)
T_B = SLOTS * DB                       # table region bytes
O_B = 128 * NT * OPB                   # offset region bytes
W_B = 128 * NT * K                     # weight region bytes
BLOB_B = T_B + O_B + W_B               # fused input bytes per core

LAST_EXEC_NS = None
LAST_RUN_WALL_S = None

_CACHE = {}

# (byte_lo, is_odd) for each of the 9 packed 12-bit R fields, relative to
# the row's R region start (byte 6): even fields j: q = b[lo] |
# ((b[lo+1] & 15) << 8); odd fields: q = (b[lo] >> 4) | (b[lo+1] << 4)
_FIELDS = [(6 + 3 * (j // 2) + (j % 2), j % 2) for j in range(9)]


def build_kernel():
    ngrp = NT // GRP
    ek = GRP * K                  # neighbor slots per partition per group
    ob_w = 5 * ek // 2            # offset bytes per partition per group
    nc = bacc.Bacc("TRN2", target_bir_lowering=False, debug=False,
                   num_devices=NCORES, num_swdge_queues=NQ)
    f16 = mybir.dt.float16
    f32 = mybir.dt.float32
    u8 = mybir.dt.uint8
    i32 = mybir.dt.int32
    blob = nc.dram_tensor("blob", [BLOB_B], u8, kind="ExternalInput").ap()
    rhs = nc.dram_tensor("rhs", [128, NT * 3], f16, kind="ExternalOutput").ap()

    with tile.TileContext(nc) as tc:
        with tc.tile_pool(name="dram", bufs=1, space="DRAM") as dpool, \
                tc.tile_pool(name="sbuf", bufs=3) as pool:
            bounce = dpool.tile([SLOTS, DB], u8, tag="bounce")
            table = dpool.tile([NPAD, DB], u8, tag="table")
            nc.sync.dma_start(
                out=bounce[:],
                in_=bass.AP(blob.tensor, 0, [(DB, SLOTS), (1, DB)]))
            nc.gpsimd.collective_compute(
                "AllGather", mybir.AluOpType.bypass,
                replica_groups=[list(range(NCORES))],
                ins=[bounce.opt()], outs=[table.opt()])

            table_ap = table[:]
            bounce_t = bounce.tensor
            bounce_off = bounce[:].offset
            qi = 0

            for g in range(ngrp):
                ob_t = pool.tile([128, ob_w], u8, tag="offb")
                o_t = pool.tile([128, ek], i32, tag="off")
                ot_t = pool.tile([128, ek], i32, tag="offt")
                on_t = pool.tile([128, ek // 2], i32, tag="offn")
                w8_t = pool.tile([128, ek], u8, tag="wgt8")
                w_t = pool.tile([128, ek], f16, tag="wgt")
                pr_t = pool.tile([128, GRP, DB], u8, tag="locpr")
                g8_t = pool.tile([128, ek, DB], u8, tag="gath")
                gr_t = pool.tile([128, ek, 9], f16, tag="grot")
                d0_t = pool.tile([128, ek], i32, tag="dec0")
                d1_t = pool.tile([128, ek], i32, tag="dec1")
                e0_t = pool.tile([128, GRP], i32, tag="edc0")
                e1_t = pool.tile([128, GRP], i32, tag="edc1")
                df_t = pool.tile([128, ek, 3], f16, tag="diff")
                m_t = pool.tile([128, ek, 3], f16, tag="macc")
                u_t = pool.tile([128, ek, 3], f16, tag="utmp")
                t2_t = pool.tile([128, GRP, 3], f32, tag="t2r")
                sv_t = pool.tile([128, GRP, 3], f32, tag="svr")
                rf_t = pool.tile([128, GRP, 9], f32, tag="rif")
                u2_t = pool.tile([128, GRP, 3], f32, tag="u2")
                o2_t = pool.tile([128, GRP, 3], f32, tag="out")
                o16_t = pool.tile([128, GRP, 3], f16, tag="out16")

                nc.sync.dma_start(
                    out=ob_t[:],
                    in_=bass.AP(blob.tensor, T_B + g * ob_w,
                                [(NT * OPB, 128), (1, ob_w)]))
                nc.sync.dma_start(
                    out=w8_t[:],
                    in_=bass.AP(blob.tensor, T_B + O_B + g * ek,
                                [(NT * K, 128), (1, ek)]))
                # local rows: bounce[p*NT + g*GRP + t] for t in [0,GRP)
                loc = bass.AP(bounce_t, bounce_off + g * GRP * DB,
                              [(NT * DB, 128), (1, GRP * DB)])
                nc.sync.dma_start(out=pr_t[:], in_=loc)

                # ids = b0 | (b1 << 8) | (nibble << 16); bit ops can't
                # cast, so cast-copy each plane to i32 first
                nc.vector.tensor_copy(out=o_t[:], in_=ob_t[:, 0:ek])
                nc.vector.tensor_copy(out=ot_t[:], in_=ob_t[:, ek:2 * ek])
                nc.vector.tensor_scalar(
                    out=ot_t[:], in0=ot_t[:], scalar1=8, scalar2=None,
                    op0=mybir.AluOpType.logical_shift_left)
                nc.vector.tensor_tensor(out=o_t[:], in0=o_t[:], in1=ot_t[:],
                                        op=mybir.AluOpType.bitwise_or)
                nc.vector.tensor_copy(
                    out=on_t[:], in_=ob_t[:, 2 * ek:2 * ek + ek // 2])

                def o_half(par):
                    return bass.AP(o_t.tensor, o_t[:].offset + par,
                                   [o_t[:].ap[0], (2, ek // 2)])
                # even edges: low nibble; odd edges: high nibble
                nc.vector.tensor_scalar(
                    out=ot_t[:, 0:ek // 2], in0=on_t[:], scalar1=15,
                    scalar2=16, op0=mybir.AluOpType.bitwise_and,
                    op1=mybir.AluOpType.logical_shift_left)
                nc.vector.tensor_tensor(
                    out=o_half(0), in0=o_half(0), in1=ot_t[:, 0:ek // 2],
                    op=mybir.AluOpType.bitwise_or)
                nc.vector.tensor_scalar(
                    out=ot_t[:, 0:ek // 2], in0=on_t[:], scalar1=4,
                    scalar2=16, op0=mybir.AluOpType.logical_shift_right,
                    op1=mybir.AluOpType.logical_shift_left)
                nc.vector.tensor_tensor(
                    out=o_half(1), in0=o_half(1), in1=ot_t[:, 0:ek // 2],
                    op=mybir.AluOpType.bitwise_or)

                # w' = u8 / 510  (cast copy, then scale in f16)
                nc.vector.tensor_copy(out=w_t[:], in_=w8_t[:])
                nc.vector.tensor_scalar(
                    out=w_t[:], in0=w_t[:], scalar1=1.0 / SW, scalar2=None,
                    op0=mybir.AluOpType.mult)

                # neighbor rows: one offset per partition per instruction,
                # rotated across the 4 SWDGE queues
                for c in range(ek):
                    bi = nc.gpsimd.indirect_dma_start(
                        out=g8_t[:, c, :], out_offset=None, in_=table_ap,
                        in_offset=bass.IndirectOffsetOnAxis(
                            ap=o_t[:, c:c + 1], axis=0))
                    if qi % NQ:
                        bi.ins.queue = f"qPoolDynamic{qi % NQ}"
                    qi += 1

                # decode a 12-bit R field from a row tile into dst[:, :, j]
                def decode_field(src_t, nrows, t0, t1, dst_view, j):
                    lo, odd = _FIELDS[j]

                    def byte(b):
                        return bass.AP(src_t.tensor, src_t[:].offset + b,
                                       [src_t[:].ap[0], (DB, nrows)])
                    if not odd:
                        nc.vector.tensor_copy(out=t0[:], in_=byte(lo))
                        nc.vector.tensor_copy(out=t1[:], in_=byte(lo + 1))
                        nc.vector.tensor_scalar(
                            out=t1[:], in0=t1[:], scalar1=15, scalar2=8,
                            op0=mybir.AluOpType.bitwise_and,
                            op1=mybir.AluOpType.logical_shift_left)
                    else:
                        nc.vector.tensor_copy(out=t0[:], in_=byte(lo))
                        nc.vector.tensor_scalar(
                            out=t0[:], in0=t0[:], scalar1=4, scalar2=None,
                            op0=mybir.AluOpType.logical_shift_right)
                        nc.vector.tensor_copy(out=t1[:], in_=byte(lo + 1))
                        nc.vector.tensor_scalar(
                            out=t1[:], in0=t1[:], scalar1=4, scalar2=None,
                            op0=mybir.AluOpType.logical_shift_left)
                    nc.vector.tensor_tensor(out=t0[:], in0=t0[:], in1=t1[:],
                                            op=mybir.AluOpType.bitwise_or)
                    # center in the INTEGER domain: f16 only holds integers
                    # <= 2048 exactly, so q-2048 converts losslessly
                    nc.vector.tensor_scalar(
                        out=t0[:], in0=t0[:], scalar1=-2048, scalar2=None,
                        op0=mybir.AluOpType.add)
                    nc.vector.tensor_copy(out=dst_view, in_=t0[:])

                # gathered R -> gr_t (f16, centered raw q - 2048)
                for j in range(9):
                    decode_field(
                        g8_t, ek, d0_t, d1_t,
                        bass.AP(gr_t.tensor, gr_t[:].offset + j,
                                [gr_t[:].ap[0], (9, ek)]), j)
                # local R -> rf_t (f32, centered)
                for j in range(9):
                    decode_field(
                        pr_t, GRP, e0_t, e1_t,
                        bass.AP(rf_t.tensor, rf_t[:].offset + j,
                                [rf_t[:].ap[0], (9, GRP)]), j)

                # p views: first 3 f16 of each row
                g16 = g8_t[:].bitcast(f16)
                gp = bass.AP(g16.tensor, g16.offset,
                             [g16.ap[0], (DB // 2, ek), (1, 3)])
                p16 = pr_t[:].bitcast(f16)
                prp = bass.AP(p16.tensor, p16.offset,
                              [p16.ap[0], (DB // 2, GRP), (0, K), (1, 3)])

                def gr_col(c):
                    # R_j rows a, column c -> [128, ek, 3]
                    return bass.AP(gr_t.tensor, gr_t[:].offset + c,
                                   [gr_t[:].ap[0], (9, ek), (3, 3)])

                def df_col(c):
                    # wd[:, e, c] broadcast over a -> [128, ek, 3]
                    return bass.AP(df_t.tensor, df_t[:].offset + c,
                                   [df_t[:].ap[0], (3, ek), (0, 3)])

                # wd = (p_i - p_j) * w'   (in place on df_t)
                nc.vector.tensor_tensor(out=df_t[:], in0=prp,
                                        in1=gp, op=mybir.AluOpType.subtract)
                wv = bass.AP(w_t.tensor, w_t[:].offset,
                             [w_t[:].ap[0], (1, ek), (0, 3)])
                nc.vector.tensor_tensor(out=df_t[:], in0=df_t[:], in1=wv,
                                        op=mybir.AluOpType.mult)

                # m = R_j @ wd  (per edge; R in raw 12-bit units)
                nc.vector.tensor_tensor(out=m_t[:], in0=gr_col(0),
                                        in1=df_col(0), op=mybir.AluOpType.mult)
                nc.vector.tensor_tensor(out=u_t[:], in0=gr_col(1),
                                        in1=df_col(1), op=mybir.AluOpType.mult)
                nc.vector.tensor_tensor(out=m_t[:], in0=m_t[:], in1=u_t[:],
                                        op=mybir.AluOpType.add)
                nc.vector.tensor_tensor(out=u_t[:], in0=gr_col(2),
                                        in1=df_col(2), op=mybir.AluOpType.mult)
                nc.vector.tensor_tensor(out=m_t[:], in0=m_t[:], in1=u_t[:],
                                        op=mybir.AluOpType.add)

                # reduce over k: t2 = sum_k m, sv = sum_k wd   (f32 accum)
                def red_view(t):
                    return bass.AP(t.tensor, t[:].offset,
                                   [t[:].ap[0], (3 * K, GRP), (1, 3), (3, K)])
                nc.vector.tensor_reduce(out=t2_t[:], in_=red_view(m_t),
                                        axis=mybir.AxisListType.X,
                                        op=mybir.AluOpType.add)
                nc.vector.tensor_reduce(out=sv_t[:], in_=red_view(df_t),
                                        axis=mybir.AxisListType.X,
                                        op=mybir.AluOpType.add)

                def rf_col(c):
                    return bass.AP(rf_t.tensor, rf_t[:].offset + c,
                                   [rf_t[:].ap[0], (9, GRP), (3, 3)])

                def sv_col(c):
                    return bass.AP(sv_t.tensor, sv_t[:].offset + c,
                                   [sv_t[:].ap[0], (3, GRP), (0, 3)])

                # term1 = R_i @ sv, then add t2 and undo the 12-bit scale
                nc.vector.tensor_tensor(out=o2_t[:], in0=rf_col(0),
                                        in1=sv_col(0), op=mybir.AluOpType.mult)
                nc.vector.tensor_tensor(out=u2_t[:], in0=rf_col(1),
                                        in1=sv_col(1), op=mybir.AluOpType.mult)
                nc.vector.tensor_tensor(out=o2_t[:], in0=o2_t[:], in1=u2_t[:],
                                        op=mybir.AluOpType.add)
                nc.vector.tensor_tensor(out=u2_t[:], in0=rf_col(2),
                                        in1=sv_col(2), op=mybir.AluOpType.mult)
                nc.vector.tensor_tensor(out=o2_t[:], in0=o2_t[:], in1=u2_t[:],
                                        op=mybir.AluOpType.add)
                nc.vector.tensor_tensor(out=o2_t[:], in0=o2_t[:], in1=t2_t[:],
                                        op=mybir.AluOpType.add)
                nc.vector.tensor_scalar(
                    out=o16_t[:], in0=o2_t[:], scalar1=1.0 / SC,
                    scalar2=None, op0=mybir.AluOpType.mult)

                nc.sync.dma_start(
                    out=rhs[:, g * GRP * 3:(g + 1) * GRP * 3], in_=o16_t[:])
    nc.compile()
    return nc


def host_stage(xyz1, neighborList, weightMatrix, rotations):
    """Build the fused [NCORES*BLOB_B] u8 upload blob (global, core-major)."""
    ngrp = NT // GRP
    ek = GRP * K
    p = np.ascontiguousarray(xyz1[0]).astype(np.float32, copy=False)
    r9 = rotations.reshape(N_FULL, 9)
    table = np.zeros((NPAD, DB), dtype=np.uint8)
    table[:N_FULL, 0:6] = p.astype(np.float16).view(np.uint8)
    q = np.empty((NPAD, 9), dtype=np.uint16)
    q[N_FULL:] = 2048
    np.add(np.clip(np.rint(r9 * SC), -2047, 2047), 2048,
           out=q[:N_FULL], casting="unsafe")
    for t in range(4):
        f0 = q[:, 2 * t]
        f1 = q[:, 2 * t + 1]
        table[:, 6 + 3 * t] = f0 & 255
        table[:, 7 + 3 * t] = (f0 >> 8) | ((f1 & 15) << 4)
        table[:, 8 + 3 * t] = f1 >> 4
    table[:, 18] = q[:, 8] & 255
    table[:, 19] = q[:, 8] >> 8
    nbr = np.zeros((NPAD, K), dtype=np.int32)
    nbr[:N_FULL] = neighborList.reshape(N_FULL, K)
    w = np.zeros((NPAD, K), dtype=np.uint8)
    np.rint(weightMatrix.reshape(N_FULL, K) * 255.0, casting="unsafe",
            out=w[:N_FULL])
    blob = np.empty((NCORES, BLOB_B), dtype=np.uint8)
    for c in range(NCORES):
        sl = slice(c * SLOTS, (c + 1) * SLOTS)
        blob[c, 0:T_B] = table[sl].ravel()
        nb = nbr[sl].reshape(128, ngrp, ek)
        b0 = (nb & 255).astype(np.uint8)
        b1 = ((nb >> 8) & 255).astype(np.uint8)
        hi = (nb >> 16).astype(np.uint8)
        nib = hi[:, :, 0::2] | (hi[:, :, 1::2] << 4)
        blob[c, T_B:T_B + O_B] = np.concatenate(
            [b0, b1, nib], axis=2).ravel()
        blob[c, T_B + O_B:] = w[sl].ravel()
    return blob.ravel()


def _make_runner(nc):
    """Like bass2jax.run_bass_via_pjrt, but the jitted executable is built
    once and reused across kernel() calls (re-tracing + XLA re-compile per
    call costs ~1s against a ~1.5s invocation)."""
    import jax
    from jax.experimental.shard_map import shard_map
    from jax.sharding import Mesh, PartitionSpec
    from concourse import bass2jax

    bass2jax.install_neuronx_cc_hook()
    partition_name = (nc.partition_id_tensor.name
                      if nc.partition_id_tensor else None)
    in_names, out_names, out_avals = [], [], []
    for alloc in nc.m.functions[0].allocations:
        if not isinstance(alloc, mybir.MemoryLocationSet):
            continue
        name = alloc.memorylocations[0].name
        if alloc.kind == "ExternalInput":
            if name != partition_name:
                in_names.append(name)
        elif alloc.kind == "ExternalOutput":
            out_names.append(name)
            out_avals.append(jax.core.ShapedArray(
                tuple(alloc.tensor_shape), mybir.dt.np(alloc.dtype)))
    assert in_names == ["blob"] and out_names == ["rhs"], (in_names, out_names)
    n_params = len(in_names)
    n_outs = len(out_avals)
    all_names = in_names + out_names
    if partition_name is not None:
        all_names.append(partition_name)

    def _body(*args):
        operands = list(args)
        if partition_name is not None:
            operands.append(bass2jax.partition_id_tensor())
        outs = bass2jax._bass_exec_p.bind(
            *operands,
            out_avals=tuple(out_avals),
            in_names=tuple(all_names),
            out_names=tuple(out_names),
            lowering_input_output_aliases=(),
            sim_require_finite=True,
            sim_require_nnan=True,
            nc=nc,
        )
        return tuple(outs)

    devices = jax.devices()[:NCORES]
    mesh = Mesh(np.asarray(devices), ("core",))
    in_specs = (PartitionSpec("core"),) * (n_params + n_outs)
    out_specs = (PartitionSpec("core"),) * n_outs
    sharded = jax.jit(
        shard_map(_body, mesh=mesh, in_specs=in_specs, out_specs=out_specs,
                  check_rep=False),
        donate_argnums=tuple(range(n_params, n_params + n_outs)),
        keep_unused=True,
    )

    def run(blob_all):
        # the kernel writes every rhs element; the donated buffer's
        # contents are irrelevant, so skip the host-side memset
        out_buf = np.empty((NCORES * 128, NT * 3), np.float16)
        out, = sharded(blob_all, out_buf)
        return np.asarray(out)

    return run


def kernel(xyz1, xyz2, neighborList, numNeighbors, accnumNeighbors,
           weightMatrix, rotations, arapWeight, trace=False):
    global LAST_EXEC_NS, LAST_RUN_WALL_S
    import time as _time
    xyz1 = np.asarray(xyz1)
    neighborList = np.asarray(neighborList)
    weightMatrix = np.asarray(weightMatrix)
    rotations = np.asarray(rotations)
    if "run" not in _CACHE:
        nc = build_kernel()
        _CACHE["run"] = _make_runner(nc)
    blob = host_stage(xyz1, neighborList, weightMatrix, rotations)
    _t0 = _time.time()
    rhs_all = _CACHE["run"](blob)
    LAST_RUN_WALL_S = _time.time() - _t0
    rhs_all = rhs_all.reshape(NCORES, 128, NT * 3)
    parts = []
    for c in range(NCORES):
        lo = c * SLOTS
        hi = min((c + 1) * SLOTS, N_FULL)
        parts.append(rhs_all[c].reshape(SLOTS, 3)[:hi - lo])
    return np.concatenate(parts, axis=0).astype(np.float32)


# revision 5
# speedup vs baseline: 1.6691x; 1.6691x over previous
"""Trainium2 Bass kernel for nn_ClosedArap (ARAP rhs, GNN message passing), v5.

rhs_i = sum_k w_ik * 0.5 * (R_i + R_j) @ (p_i - p_j),  j = nbr[i, k]
      = R_i @ (sum_k w'_ik d_ik) + sum_k R_j @ (w'_ik d_ik),   w' = w/2

The axon link to the device runs at ~35 MB/s serialized, so a device
invocation's cost is dominated by uploaded bytes.  This version ships:
  - per-vertex table rows packed to 20B: p as 3x f16 (6B) + R as 9x
    12-bit fixed point (14B, scale 346.9, offset 2048); only a 1/8 shard
    is uploaded per core and an on-device AllGather replicates the full
    table into each core's DRAM,
  - neighbor ids packed to 20 bits (2.5B/edge: two byte planes + a
    nibble plane),
  - weights quantized to u8 (w' = u8/510, descaled at load),
  - everything fused into one u8 blob per core (one link transfer),
  - local rows streamed from the core's own shard (no upload),
  - rhs downloaded as f16.
The 12-bit fields are decoded on DVE with shift/and/or into f16 (offset
-2048 applied in f16; values stay exact integers <= 2048).  rhs is linear
in R, so the 1/346.9 scale folds into the final output copy.  Total rel
err vs the f32 reference ~1.7e-3 (threshold 2e-2).

Gathers are one-offset-per-partition indirect DMAs (the multi-offset
"vector dynamic offset" DMA form generates descriptors for only one
partition in this stack), rotated over 4 SWDGE queues so the per-queue
16-bit DMA-semaphore fields stay under 65535 in a single-NEFF invocation.

Slot map: core c owns vertex slots [c*SLOTS, (c+1)*SLOTS), slot (p, t)
of core c holds vertex c*SLOTS + p*NT + t; host staging is pure reshape
of [NPAD]-padded arrays.
"""
import numpy as np

from concourse import bass, bacc, mybir, tile

N_FULL = 1_000_000
K = 8
NCORES = 8
DB = 20           # packed row bytes: p 3xf16 (6) + R 9x12bit (14)
GRP = 16          # vertex tiles (of 128) per pipeline group
NT = 992          # vertex tiles per core; 128*992 = 126976 slots
SLOTS = 128 * NT
NPAD = NCORES * SLOTS          # 1015808 padded table rows (< 2^20)
NQ = 4            # SWDGE queues to rotate gathers over
SW = 510.0        # weight u8 scale (w' = w/2 = u8/510)
SC = 346.9        # rotation 12-bit scale (clips beyond +-5.9 sigma)
OPB = 5 * K // 2  # offset bytes per vertex slot (2.5 per edge)
T_B = SLOTS * DB                       # table region bytes
O_B = 128 * NT * OPB                   # offset region bytes
W_B = 128 * NT * K                     # weight region bytes
BLOB_B = T_B + O_B + W_B               # fused input bytes per core

LAST_EXEC_NS = None
LAST_RUN_WALL_S = None

_CACHE = {}

# (byte_lo, is_odd) for each of the 9 packed 12-bit R fields, relative to
# the row's R region start (byte 6): even fields j: q = b[lo] |
# ((b[lo+1] & 15) << 8); odd fields: q = (b[lo] >> 4) | (b[lo+1] << 4)
_FIELDS = [(6 + 3 * (j // 2) + (j % 2), j % 2) for j in range(9)]


def build_kernel():
    ngrp = NT // GRP
    ek = GRP * K                  # neighbor slots per partition per group
    ob_w = 5 * ek // 2            # offset bytes per partition per group
    nc = bacc.Bacc("TRN2", target_bir_lowering=False, debug=False,
                   num_devices=NCORES, num_swdge_queues=NQ)
    f16 = mybir.dt.float16
    f32 = mybir.dt.float32
    u8 = mybir.dt.uint8
    i32 = mybir.dt.int32
    blob = nc.dram_tensor("blob", [BLOB_B], u8, kind="ExternalInput").ap()
    rhs = nc.dram_tensor("rhs", [128, NT * 3], f16, kind="ExternalOutput").ap()

    with tile.TileContext(nc) as tc:
        with tc.tile_pool(name="dram", bufs=1, space="DRAM") as dpool, \
                tc.tile_pool(name="sbuf", bufs=3) as pool:
            bounce = dpool.tile([SLOTS, DB], u8, tag="bounce")
            table = dpool.tile([NPAD, DB], u8, tag="table")
            nc.sync.dma_start(
                out=bounce[:],
                in_=bass.AP(blob.tensor, 0, [(DB, SLOTS), (1, DB)]))
            nc.gpsimd.collective_compute(
                "AllGather", mybir.AluOpType.bypass,
                replica_groups=[list(range(NCORES))],
                ins=[bounce.opt()], outs=[table.opt()])

            table_ap = table[:]
            bounce_t = bounce.tensor
            bounce_off = bounce[:].offset
            qi = 0

            for g in range(ngrp):
                ob_t = pool.tile([128, ob_w], u8, tag="offb")
                o_t = pool.tile([128, ek], i32, tag="off")
                ot_t = pool.tile([128, ek], i32, tag="offt")
                on_t = pool.tile([128, ek // 2], i32, tag="offn")
                w8_t = pool.tile([128, ek], u8, tag="wgt8")
                w_t = pool.tile([128, ek], f16, tag="wgt")
                pr_t = pool.tile([128, GRP, DB], u8, tag="locpr")
                g8_t = pool.tile([128, ek, DB], u8, tag="gath")
                gr_t = pool.tile([128, ek, 9], f16, tag="grot")
                d0_t = pool.tile([128, ek], i32, tag="dec0")
                d1_t = pool.tile([128, ek], i32, tag="dec1")
                e0_t = pool.tile([128, GRP], i32, tag="edc0")
                e1_t = pool.tile([128, GRP], i32, tag="edc1")
                df_t = pool.tile([128, ek, 3], f16, tag="diff")
                m_t = pool.tile([128, ek, 3], f16, tag="macc")
                u_t = pool.tile([128, ek, 3], f16, tag="utmp")
                t2_t = pool.tile([128, GRP, 3], f32, tag="t2r")
                sv_t = pool.tile([128, GRP, 3], f32, tag="svr")
                rf_t = pool.tile([128, GRP, 9], f32, tag="rif")
                u2_t = pool.tile([128, GRP, 3], f32, tag="u2")
                o2_t = pool.tile([128, GRP, 3], f32, tag="out")
                o16_t = pool.tile([128, GRP, 3], f16, tag="out16")

                nc.sync.dma_start(
                    out=ob_t[:],
                    in_=bass.AP(blob.tensor, T_B + g * ob_w,
                                [(NT * OPB, 128), (1, ob_w)]))
                nc.sync.dma_start(
                    out=w8_t[:],
                    in_=bass.AP(blob.tensor, T_B + O_B + g * ek,
                                [(NT * K, 128), (1, ek)]))
                # local rows: bounce[p*NT + g*GRP + t] for t in [0,GRP)
                loc = bass.AP(bounce_t, bounce_off + g * GRP * DB,
                              [(NT * DB, 128), (1, GRP * DB)])
                nc.sync.dma_start(out=pr_t[:], in_=loc)

                # ids = b0 | (b1 << 8) | (nibble << 16); bit ops can't
                # cast, so cast-copy each plane to i32 first
                nc.vector.tensor_copy(out=o_t[:], in_=ob_t[:, 0:ek])
                nc.vector.tensor_copy(out=ot_t[:], in_=ob_t[:, ek:2 * ek])
                nc.vector.tensor_scalar(
                    out=ot_t[:], in0=ot_t[:], scalar1=8, scalar2=None,
                    op0=mybir.AluOpType.logical_shift_left)
                nc.vector.tensor_tensor(out=o_t[:], in0=o_t[:], in1=ot_t[:],
                                        op=mybir.AluOpType.bitwise_or)
                nc.vector.tensor_copy(
                    out=on_t[:], in_=ob_t[:, 2 * ek:2 * ek + ek // 2])

                def o_half(par):
                    return bass.AP(o_t.tensor, o_t[:].offset + par,
                                   [o_t[:].ap[0], (2, ek // 2)])
                # even edges: low nibble; odd edges: high nibble
                nc.vector.tensor_scalar(
                    out=ot_t[:, 0:ek // 2], in0=on_t[:], scalar1=15,
                    scalar2=16, op0=mybir.AluOpType.bitwise_and,
                    op1=mybir.AluOpType.logical_shift_left)
                nc.vector.tensor_tensor(
                    out=o_half(0), in0=o_half(0), in1=ot_t[:, 0:ek // 2],
                    op=mybir.AluOpType.bitwise_or)
                nc.vector.tensor_scalar(
                    out=ot_t[:, 0:ek // 2], in0=on_t[:], scalar1=4,
                    scalar2=16, op0=mybir.AluOpType.logical_shift_right,
                    op1=mybir.AluOpType.logical_shift_left)
                nc.vector.tensor_tensor(
                    out=o_half(1), in0=o_half(1), in1=ot_t[:, 0:ek // 2],
                    op=mybir.AluOpType.bitwise_or)

                # w' = u8 / 510  (cast copy, then scale in f16)
                nc.vector.tensor_copy(out=w_t[:], in_=w8_t[:])
                nc.vector.tensor_scalar(
                    out=w_t[:], in0=w_t[:], scalar1=1.0 / SW, scalar2=None,
                    op0=mybir.AluOpType.mult)

                # neighbor rows: one offset per partition per instruction,
                # rotated across the 4 SWDGE queues
                for c in range(ek):
                    bi = nc.gpsimd.indirect_dma_start(
                        out=g8_t[:, c, :], out_offset=None, in_=table_ap,
                        in_offset=bass.IndirectOffsetOnAxis(
                            ap=o_t[:, c:c + 1], axis=0))
                    if qi % NQ:
                        bi.ins.queue = f"qPoolDynamic{qi % NQ}"
                    qi += 1

                # decode a 12-bit R field from a row tile into dst[:, :, j]
                def decode_field(src_t, nrows, t0, t1, dst_view, j):
                    lo, odd = _FIELDS[j]

                    def byte(b):
                        return bass.AP(src_t.tensor, src_t[:].offset + b,
                                       [src_t[:].ap[0], (DB, nrows)])
                    if not odd:
                        nc.vector.tensor_copy(out=t0[:], in_=byte(lo))
                        nc.vector.tensor_copy(out=t1[:], in_=byte(lo + 1))
                        nc.vector.tensor_scalar(
                            out=t1[:], in0=t1[:], scalar1=15, scalar2=8,
                            op0=mybir.AluOpType.bitwise_and,
                            op1=mybir.AluOpType.logical_shift_left)
                    else:
                        nc.vector.tensor_copy(out=t0[:], in_=byte(lo))
                        nc.vector.tensor_scalar(
                            out=t0[:], in0=t0[:], scalar1=4, scalar2=None,
                            op0=mybir.AluOpType.logical_shift_right)
                        nc.vector.tensor_copy(out=t1[:], in_=byte(lo + 1))
                        nc.vector.tensor_scalar(
                            out=t1[:], in0=t1[:], scalar1=4, scalar2=None,
                            op0=mybir.AluOpType.logical_shift_left)
                    nc.vector.tensor_tensor(out=t0[:], in0=t0[:], in1=t1[:],
                                            op=mybir.AluOpType.bitwise_or)
                    # center in the INTEGER domain: f16 only holds integers
                    # <= 2048 exactly, so q-2048 converts losslessly
                    nc.vector.tensor_scalar(
                        out=t0[:], in0=t0[:], scalar1=-2048, scalar2=None,
                        op0=mybir.AluOpType.add)
                    nc.vector.tensor_copy(out=dst_view, in_=t0[:])

                # gathered R -> gr_t (f16, centered raw q - 2048)
                for j in range(9):
                    decode_field(
                        g8_t, ek, d0_t, d1_t,
                        bass.AP(gr_t.tensor, gr_t[:].offset + j,
                                [gr_t[:].ap[0], (9, ek)]), j)
                # local R -> rf_t (f32, centered)
                for j in range(9):
                    decode_field(
                        pr_t, GRP, e0_t, e1_t,
                        bass.AP(rf_t.tensor, rf_t[:].offset + j,
                                [rf_t[:].ap[0], (9, GRP)]), j)

                # p views: first 3 f16 of each row
                g16 = g8_t[:].bitcast(f16)
                gp = bass.AP(g16.tensor, g16.offset,
                             [g16.ap[0], (DB // 2, ek), (1, 3)])
                p16 = pr_t[:].bitcast(f16)
                prp = bass.AP(p16.tensor, p16.offset,
                              [p16.ap[0], (DB // 2, GRP), (0, K), (1, 3)])

                def gr_col(c):
                    # R_j rows a, column c -> [128, ek, 3]
                    return bass.AP(gr_t.tensor, gr_t[:].offset + c,
                                   [gr_t[:].ap[0], (9, ek), (3, 3)])

                def df_col(c):
                    # wd[:, e, c] broadcast over a -> [128, ek, 3]
                    return bass.AP(df_t.tensor, df_t[:].offset + c,
                                   [df_t[:].ap[0], (3, ek), (0, 3)])

                # wd = (p_i - p_j) * w'   (in place on df_t)
                nc.vector.tensor_tensor(out=df_t[:], in0=prp,
                                        in1=gp, op=mybir.AluOpType.subtract)
                wv = bass.AP(w_t.tensor, w_t[:].offset,
                             [w_t[:].ap[0], (1, ek), (0, 3)])
                nc.vector.tensor_tensor(out=df_t[:], in0=df_t[:], in1=wv,
                                        op=mybir.AluOpType.mult)

                # m = R_j @ wd  (per edge; R in raw 12-bit units)
                nc.vector.tensor_tensor(out=m_t[:], in0=gr_col(0),
                                        in1=df_col(0), op=mybir.AluOpType.mult)
                nc.vector.tensor_tensor(out=u_t[:], in0=gr_col(1),
                                        in1=df_col(1), op=mybir.AluOpType.mult)
                nc.vector.tensor_tensor(out=m_t[:], in0=m_t[:], in1=u_t[:],
                                        op=mybir.AluOpType.add)
                nc.vector.tensor_tensor(out=u_t[:], in0=gr_col(2),
                                        in1=df_col(2), op=mybir.AluOpType.mult)
                nc.vector.tensor_tensor(out=m_t[:], in0=m_t[:], in1=u_t[:],
                                        op=mybir.AluOpType.add)

                # reduce over k: t2 = sum_k m, sv = sum_k wd   (f32 accum)
                def red_view(t):
                    return bass.AP(t.tensor, t[:].offset,
                                   [t[:].ap[0], (3 * K, GRP), (1, 3), (3, K)])
                nc.vector.tensor_reduce(out=t2_t[:], in_=red_view(m_t),
                                        axis=mybir.AxisListType.X,
                                        op=mybir.AluOpType.add)
                nc.vector.tensor_reduce(out=sv_t[:], in_=red_view(df_t),
                                        axis=mybir.AxisListType.X,
                                        op=mybir.AluOpType.add)

                def rf_col(c):
                    return bass.AP(rf_t.tensor, rf_t[:].offset + c,
                                   [rf_t[:].ap[0], (9, GRP), (3, 3)])

                def sv_col(c):
                    return bass.AP(sv_t.tensor, sv_t[:].offset + c,
                                   [sv_t[:].ap[0], (3, GRP), (0, 3)])

                # term1 = R_i @ sv, then add t2 and undo the 12-bit scale
                nc.vector.tensor_tensor(out=o2_t[:], in0=rf_col(0),
                                        in1=sv_col(0), op=mybir.AluOpType.mult)
                nc.vector.tensor_tensor(out=u2_t[:], in0=rf_col(1),
                                        in1=sv_col(1), op=mybir.AluOpType.mult)
                nc.vector.tensor_tensor(out=o2_t[:], in0=o2_t[:], in1=u2_t[:],
                                        op=mybir.AluOpType.add)
                nc.vector.tensor_tensor(out=u2_t[:], in0=rf_col(2),
                                        in1=sv_col(2), op=mybir.AluOpType.mult)
                nc.vector.tensor_tensor(out=o2_t[:], in0=o2_t[:], in1=u2_t[:],
                                        op=mybir.AluOpType.add)
                nc.vector.tensor_tensor(out=o2_t[:], in0=o2_t[:], in1=t2_t[:],
                                        op=mybir.AluOpType.add)
                nc.vector.tensor_scalar(
                    out=o16_t[:], in0=o2_t[:], scalar1=1.0 / SC,
                    scalar2=None, op0=mybir.AluOpType.mult)

                nc.sync.dma_start(
                    out=rhs[:, g * GRP * 3:(g + 1) * GRP * 3], in_=o16_t[:])
    nc.compile()
    return nc


def host_stage(xyz1, neighborList, weightMatrix, rotations):
    """Build the fused [NCORES*BLOB_B] u8 upload blob (global, core-major)."""
    ngrp = NT // GRP
    ek = GRP * K
    p = np.ascontiguousarray(xyz1[0]).astype(np.float32, copy=False)
    r9 = rotations.reshape(N_FULL, 9)
    table = np.zeros((NPAD, DB), dtype=np.uint8)
    table[:N_FULL, 0:6] = p.astype(np.float16).view(np.uint8)
    q = np.empty((NPAD, 9), dtype=np.uint16)
    q[N_FULL:] = 2048
    np.add(np.clip(np.rint(r9 * SC), -2047, 2047), 2048,
           out=q[:N_FULL], casting="unsafe")
    for t in range(4):
        f0 = q[:, 2 * t]
        f1 = q[:, 2 * t + 1]
        table[:, 6 + 3 * t] = f0 & 255
        table[:, 7 + 3 * t] = (f0 >> 8) | ((f1 & 15) << 4)
        table[:, 8 + 3 * t] = f1 >> 4
    table[:, 18] = q[:, 8] & 255
    table[:, 19] = q[:, 8] >> 8
    nbr = np.zeros((NPAD, K), dtype=np.int32)
    nbr[:N_FULL] = neighborList.reshape(N_FULL, K)
    w = np.zeros((NPAD, K), dtype=np.uint8)
    np.rint(weightMatrix.reshape(N_FULL, K) * 255.0, casting="unsafe",
            out=w[:N_FULL])
    blob = np.empty((NCORES, BLOB_B), dtype=np.uint8)
    for c in range(NCORES):
        sl = slice(c * SLOTS, (c + 1) * SLOTS)
        blob[c, 0:T_B] = table[sl].ravel()
        nb = nbr[sl].reshape(128, ngrp, ek)
        b0 = (nb & 255).astype(np.uint8)
        b1 = ((nb >> 8) & 255).astype(np.uint8)
        hi = (nb >> 16).astype(np.uint8)
        nib = hi[:, :, 0::2] | (hi[:, :, 1::2] << 4)
        blob[c, T_B:T_B + O_B] = np.concatenate(
            [b0, b1, nib], axis=2).ravel()
        blob[c, T_B + O_B:] = w[sl].ravel()
    return blob.ravel()


def _make_runner(nc):
    """Like bass2jax.run_bass_via_pjrt, but the jitted executable is built
    once and reused across kernel() calls (re-tracing + XLA re-compile per
    call costs ~1s against a ~1.5s invocation)."""
    import jax
    from jax.experimental.shard_map import shard_map
    from jax.sharding import Mesh, PartitionSpec
    from concourse import bass2jax

    bass2jax.install_neuronx_cc_hook()
    partition_name = (nc.partition_id_tensor.name
                      if nc.partition_id_tensor else None)
    in_names, out_names, out_avals = [], [], []
    for alloc in nc.m.functions[0].allocations:
        if not isinstance(alloc, mybir.MemoryLocationSet):
            continue
        name = alloc.memorylocations[0].name
        if alloc.kind == "ExternalInput":
            if name != partition_name:
                in_names.append(name)
        elif alloc.kind == "ExternalOutput":
            out_names.append(name)
            out_avals.append(jax.core.ShapedArray(
                tuple(alloc.tensor_shape), mybir.dt.np(alloc.dtype)))
    assert in_names == ["blob"] and out_names == ["rhs"], (in_names, out_names)
    n_params = len(in_names)
    n_outs = len(out_avals)
    all_names = in_names + out_names
    if partition_name is not None:
        all_names.append(partition_name)

    def _body(*args):
        operands = list(args)
        if partition_name is not None:
            operands.append(bass2jax.partition_id_tensor())
        outs = bass2jax._bass_exec_p.bind(
            *operands,
            out_avals=tuple(out_avals),
            in_names=tuple(all_names),
            out_names=tuple(out_names),
            lowering_input_output_aliases=(),
            sim_require_finite=True,
            sim_require_nnan=True,
            nc=nc,
        )
        return tuple(outs)

    devices = jax.devices()[:NCORES]
    mesh = Mesh(np.asarray(devices), ("core",))
    in_specs = (PartitionSpec("core"),) * (n_params + n_outs)
    out_specs = (PartitionSpec("core"),) * n_outs
    sharded = jax.jit(
        shard_map(_body, mesh=mesh, in_specs=in_specs, out_specs=out_specs,
                  check_rep=False),
        donate_argnums=tuple(range(n_params, n_params + n_outs)),
        keep_unused=True,
    )

    def run(blob_all):
        # The kernel writes every rhs element, so the donated buffer's
        # contents are irrelevant.  Steady state donates the PREVIOUS
        # call's device-resident output array -- no 6 MB host upload for
        # the scratch buffer after the first call.
        buf = _CACHE.get("outbuf")
        if buf is None:
            buf = np.empty((NCORES * 128, NT * 3), np.float16)
        out, = sharded(blob_all, buf)
        _CACHE["outbuf"] = out
        return np.asarray(out)

    return run


def kernel(xyz1, xyz2, neighborList, numNeighbors, accnumNeighbors,
           weightMatrix, rotations, arapWeight, trace=False):
    global LAST_EXEC_NS, LAST_RUN_WALL_S
    import time as _time
    xyz1 = np.asarray(xyz1)
    neighborList = np.asarray(neighborList)
    weightMatrix = np.asarray(weightMatrix)
    rotations = np.asarray(rotations)
    if "run" not in _CACHE:
        nc = build_kernel()
        _CACHE["run"] = _make_runner(nc)
    blob = host_stage(xyz1, neighborList, weightMatrix, rotations)
    _t0 = _time.time()
    rhs_all = _CACHE["run"](blob)
    LAST_RUN_WALL_S = _time.time() - _t0
    rhs_all = rhs_all.reshape(NCORES, 128, NT * 3)
    parts = []
    for c in range(NCORES):
        lo = c * SLOTS
        hi = min((c + 1) * SLOTS, N_FULL)
        parts.append(rhs_all[c].reshape(SLOTS, 3)[:hi - lo])
    return np.concatenate(parts, axis=0).astype(np.float32)


# revision 6
# speedup vs baseline: 2.6796x; 1.6054x over previous
"""Trainium2 Bass kernel for nn_ClosedArap (ARAP rhs, GNN message passing), v5.

rhs_i = sum_k w_ik * 0.5 * (R_i + R_j) @ (p_i - p_j),  j = nbr[i, k]
      = R_i @ (sum_k w'_ik d_ik) + sum_k R_j @ (w'_ik d_ik),   w' = w/2

The axon link to the device runs at ~35 MB/s serialized, so a device
invocation's cost is dominated by uploaded bytes.  This version ships:
  - per-vertex table rows packed to 20B: p as 3x f16 (6B) + R as 9x
    12-bit fixed point (14B, scale 346.9, offset 2048); only a 1/8 shard
    is uploaded per core and an on-device AllGather replicates the full
    table into each core's DRAM,
  - neighbor ids packed to 20 bits (2.5B/edge: two byte planes + a
    nibble plane),
  - weights quantized to u8 (w' = u8/510, descaled at load),
  - everything fused into one u8 blob per core (one link transfer),
  - local rows streamed from the core's own shard (no upload),
  - rhs downloaded as f16.
The 12-bit fields are decoded on DVE with shift/and/or into f16 (offset
-2048 applied in f16; values stay exact integers <= 2048).  rhs is linear
in R, so the 1/346.9 scale folds into the final output copy.  Total rel
err vs the f32 reference ~1.7e-3 (threshold 2e-2).

Gathers are one-offset-per-partition indirect DMAs (the multi-offset
"vector dynamic offset" DMA form generates descriptors for only one
partition in this stack), rotated over 4 SWDGE queues so the per-queue
16-bit DMA-semaphore fields stay under 65535 in a single-NEFF invocation.

Slot map: core c owns vertex slots [c*SLOTS, (c+1)*SLOTS), slot (p, t)
of core c holds vertex c*SLOTS + p*NT + t; host staging is pure reshape
of [NPAD]-padded arrays.
"""
import numpy as np

from concourse import bass, bacc, mybir, tile

N_FULL = 1_000_000
K = 8
NCORES = 8
DB = 20           # packed row bytes: p 3xf16 (6) + R 9x12bit (14)
GRP = 16          # vertex tiles (of 128) per pipeline group
NT = 992          # vertex tiles per core; 128*992 = 126976 slots
SLOTS = 128 * NT
NPAD = NCORES * SLOTS          # 1015808 padded table rows (< 2^20)
NQ = 4            # SWDGE queues to rotate gathers over
SW = 510.0        # weight u8 scale (w' = w/2 = u8/510)
SC = 346.9        # rotation 12-bit scale (clips beyond +-5.9 sigma)
OPB = 5 * K // 2  # offset bytes per vertex slot (2.5 per edge)
T_B = SLOTS * DB                       # table region bytes
O_B = 128 * NT * OPB                   # offset region bytes
W_B = 128 * NT * K                     # weight region bytes
BLOB_B = T_B + O_B + W_B               # fused input bytes per core

LAST_EXEC_NS = None
LAST_RUN_WALL_S = None

_CACHE = {}

# (byte_lo, is_odd) for each of the 9 packed 12-bit R fields, relative to
# the row's R region start (byte 6): even fields j: q = b[lo] |
# ((b[lo+1] & 15) << 8); odd fields: q = (b[lo] >> 4) | (b[lo+1] << 4)
_FIELDS = [(6 + 3 * (j // 2) + (j % 2), j % 2) for j in range(9)]


def build_kernel():
    ngrp = NT // GRP
    ek = GRP * K                  # neighbor slots per partition per group
    ob_w = 5 * ek // 2            # offset bytes per partition per group
    nc = bacc.Bacc("TRN2", target_bir_lowering=False, debug=False,
                   num_devices=NCORES, num_swdge_queues=NQ)
    f16 = mybir.dt.float16
    f32 = mybir.dt.float32
    u8 = mybir.dt.uint8
    i32 = mybir.dt.int32
    blob = nc.dram_tensor("blob", [BLOB_B], u8, kind="ExternalInput").ap()
    rhs = nc.dram_tensor("rhs", [128, NT * 3], f16, kind="ExternalOutput").ap()

    with tile.TileContext(nc) as tc:
        with tc.tile_pool(name="dram", bufs=1, space="DRAM") as dpool, \
                tc.tile_pool(name="sbuf", bufs=3) as pool:
            bounce = dpool.tile([SLOTS, DB], u8, tag="bounce")
            table = dpool.tile([NPAD, DB], u8, tag="table")
            nc.sync.dma_start(
                out=bounce[:],
                in_=bass.AP(blob.tensor, 0, [(DB, SLOTS), (1, DB)]))
            nc.gpsimd.collective_compute(
                "AllGather", mybir.AluOpType.bypass,
                replica_groups=[list(range(NCORES))],
                ins=[bounce.opt()], outs=[table.opt()])

            table_ap = table[:]
            bounce_t = bounce.tensor
            bounce_off = bounce[:].offset
            qi = 0

            for g in range(ngrp):
                ob_t = pool.tile([128, ob_w], u8, tag="offb")
                o_t = pool.tile([128, ek], i32, tag="off")
                ot_t = pool.tile([128, ek], i32, tag="offt")
                on_t = pool.tile([128, ek // 2], i32, tag="offn")
                w8_t = pool.tile([128, ek], u8, tag="wgt8")
                w_t = pool.tile([128, ek], f16, tag="wgt")
                pr_t = pool.tile([128, GRP, DB], u8, tag="locpr")
                g8_t = pool.tile([128, ek, DB], u8, tag="gath")
                gr_t = pool.tile([128, ek, 9], f16, tag="grot")
                d0_t = pool.tile([128, ek], i32, tag="dec0")
                d1_t = pool.tile([128, ek], i32, tag="dec1")
                e0_t = pool.tile([128, GRP], i32, tag="edc0")
                e1_t = pool.tile([128, GRP], i32, tag="edc1")
                df_t = pool.tile([128, ek, 3], f16, tag="diff")
                m_t = pool.tile([128, ek, 3], f16, tag="macc")
                u_t = pool.tile([128, ek, 3], f16, tag="utmp")
                t2_t = pool.tile([128, GRP, 3], f32, tag="t2r")
                sv_t = pool.tile([128, GRP, 3], f32, tag="svr")
                rf_t = pool.tile([128, GRP, 9], f32, tag="rif")
                u2_t = pool.tile([128, GRP, 3], f32, tag="u2")
                o2_t = pool.tile([128, GRP, 3], f32, tag="out")
                o16_t = pool.tile([128, GRP, 3], f16, tag="out16")

                nc.sync.dma_start(
                    out=ob_t[:],
                    in_=bass.AP(blob.tensor, T_B + g * ob_w,
                                [(NT * OPB, 128), (1, ob_w)]))
                nc.sync.dma_start(
                    out=w8_t[:],
                    in_=bass.AP(blob.tensor, T_B + O_B + g * ek,
                                [(NT * K, 128), (1, ek)]))
                # local rows: bounce[p*NT + g*GRP + t] for t in [0,GRP)
                loc = bass.AP(bounce_t, bounce_off + g * GRP * DB,
                              [(NT * DB, 128), (1, GRP * DB)])
                nc.sync.dma_start(out=pr_t[:], in_=loc)

                # ids = b0 | (b1 << 8) | (nibble << 16); bit ops can't
                # cast, so cast-copy each plane to i32 first
                nc.vector.tensor_copy(out=o_t[:], in_=ob_t[:, 0:ek])
                nc.vector.tensor_copy(out=ot_t[:], in_=ob_t[:, ek:2 * ek])
                nc.vector.tensor_scalar(
                    out=ot_t[:], in0=ot_t[:], scalar1=8, scalar2=None,
                    op0=mybir.AluOpType.logical_shift_left)
                nc.vector.tensor_tensor(out=o_t[:], in0=o_t[:], in1=ot_t[:],
                                        op=mybir.AluOpType.bitwise_or)
                nc.vector.tensor_copy(
                    out=on_t[:], in_=ob_t[:, 2 * ek:2 * ek + ek // 2])

                def o_half(par):
                    return bass.AP(o_t.tensor, o_t[:].offset + par,
                                   [o_t[:].ap[0], (2, ek // 2)])
                # even edges: low nibble; odd edges: high nibble
                nc.vector.tensor_scalar(
                    out=ot_t[:, 0:ek // 2], in0=on_t[:], scalar1=15,
                    scalar2=16, op0=mybir.AluOpType.bitwise_and,
                    op1=mybir.AluOpType.logical_shift_left)
                nc.vector.tensor_tensor(
                    out=o_half(0), in0=o_half(0), in1=ot_t[:, 0:ek // 2],
                    op=mybir.AluOpType.bitwise_or)
                nc.vector.tensor_scalar(
                    out=ot_t[:, 0:ek // 2], in0=on_t[:], scalar1=4,
                    scalar2=16, op0=mybir.AluOpType.logical_shift_right,
                    op1=mybir.AluOpType.logical_shift_left)
                nc.vector.tensor_tensor(
                    out=o_half(1), in0=o_half(1), in1=ot_t[:, 0:ek // 2],
                    op=mybir.AluOpType.bitwise_or)

                # w' = u8 / 510  (cast copy, then scale in f16)
                nc.vector.tensor_copy(out=w_t[:], in_=w8_t[:])
                nc.vector.tensor_scalar(
                    out=w_t[:], in0=w_t[:], scalar1=1.0 / SW, scalar2=None,
                    op0=mybir.AluOpType.mult)

                # neighbor rows: one offset per partition per instruction,
                # rotated across the 4 SWDGE queues
                for c in range(ek):
                    bi = nc.gpsimd.indirect_dma_start(
                        out=g8_t[:, c, :], out_offset=None, in_=table_ap,
                        in_offset=bass.IndirectOffsetOnAxis(
                            ap=o_t[:, c:c + 1], axis=0))
                    if qi % NQ:
                        bi.ins.queue = f"qPoolDynamic{qi % NQ}"
                    qi += 1

                # decode a 12-bit R field from a row tile into dst[:, :, j]
                def decode_field(src_t, nrows, t0, t1, dst_view, j):
                    lo, odd = _FIELDS[j]

                    def byte(b):
                        return bass.AP(src_t.tensor, src_t[:].offset + b,
                                       [src_t[:].ap[0], (DB, nrows)])
                    if not odd:
                        nc.vector.tensor_copy(out=t0[:], in_=byte(lo))
                        nc.vector.tensor_copy(out=t1[:], in_=byte(lo + 1))
                        nc.vector.tensor_scalar(
                            out=t1[:], in0=t1[:], scalar1=15, scalar2=8,
                            op0=mybir.AluOpType.bitwise_and,
                            op1=mybir.AluOpType.logical_shift_left)
                    else:
                        nc.vector.tensor_copy(out=t0[:], in_=byte(lo))
                        nc.vector.tensor_scalar(
                            out=t0[:], in0=t0[:], scalar1=4, scalar2=None,
                            op0=mybir.AluOpType.logical_shift_right)
                        nc.vector.tensor_copy(out=t1[:], in_=byte(lo + 1))
                        nc.vector.tensor_scalar(
                            out=t1[:], in0=t1[:], scalar1=4, scalar2=None,
                            op0=mybir.AluOpType.logical_shift_left)
                    nc.vector.tensor_tensor(out=t0[:], in0=t0[:], in1=t1[:],
                                            op=mybir.AluOpType.bitwise_or)
                    # center in the INTEGER domain: f16 only holds integers
                    # <= 2048 exactly, so q-2048 converts losslessly
                    nc.vector.tensor_scalar(
                        out=t0[:], in0=t0[:], scalar1=-2048, scalar2=None,
                        op0=mybir.AluOpType.add)
                    nc.vector.tensor_copy(out=dst_view, in_=t0[:])

                # gathered R -> gr_t (f16, centered raw q - 2048)
                for j in range(9):
                    decode_field(
                        g8_t, ek, d0_t, d1_t,
                        bass.AP(gr_t.tensor, gr_t[:].offset + j,
                                [gr_t[:].ap[0], (9, ek)]), j)
                # local R -> rf_t (f32, centered)
                for j in range(9):
                    decode_field(
                        pr_t, GRP, e0_t, e1_t,
                        bass.AP(rf_t.tensor, rf_t[:].offset + j,
                                [rf_t[:].ap[0], (9, GRP)]), j)

                # p views: first 3 f16 of each row
                g16 = g8_t[:].bitcast(f16)
                gp = bass.AP(g16.tensor, g16.offset,
                             [g16.ap[0], (DB // 2, ek), (1, 3)])
                p16 = pr_t[:].bitcast(f16)
                prp = bass.AP(p16.tensor, p16.offset,
                              [p16.ap[0], (DB // 2, GRP), (0, K), (1, 3)])

                def gr_col(c):
                    # R_j rows a, column c -> [128, ek, 3]
                    return bass.AP(gr_t.tensor, gr_t[:].offset + c,
                                   [gr_t[:].ap[0], (9, ek), (3, 3)])

                def df_col(c):
                    # wd[:, e, c] broadcast over a -> [128, ek, 3]
                    return bass.AP(df_t.tensor, df_t[:].offset + c,
                                   [df_t[:].ap[0], (3, ek), (0, 3)])

                # wd = (p_i - p_j) * w'   (in place on df_t)
                nc.vector.tensor_tensor(out=df_t[:], in0=prp,
                                        in1=gp, op=mybir.AluOpType.subtract)
                wv = bass.AP(w_t.tensor, w_t[:].offset,
                             [w_t[:].ap[0], (1, ek), (0, 3)])
                nc.vector.tensor_tensor(out=df_t[:], in0=df_t[:], in1=wv,
                                        op=mybir.AluOpType.mult)

                # m = R_j @ wd  (per edge; R in raw 12-bit units)
                nc.vector.tensor_tensor(out=m_t[:], in0=gr_col(0),
                                        in1=df_col(0), op=mybir.AluOpType.mult)
                nc.vector.tensor_tensor(out=u_t[:], in0=gr_col(1),
                                        in1=df_col(1), op=mybir.AluOpType.mult)
                nc.vector.tensor_tensor(out=m_t[:], in0=m_t[:], in1=u_t[:],
                                        op=mybir.AluOpType.add)
                nc.vector.tensor_tensor(out=u_t[:], in0=gr_col(2),
                                        in1=df_col(2), op=mybir.AluOpType.mult)
                nc.vector.tensor_tensor(out=m_t[:], in0=m_t[:], in1=u_t[:],
                                        op=mybir.AluOpType.add)

                # reduce over k: t2 = sum_k m, sv = sum_k wd   (f32 accum)
                def red_view(t):
                    return bass.AP(t.tensor, t[:].offset,
                                   [t[:].ap[0], (3 * K, GRP), (1, 3), (3, K)])
                nc.vector.tensor_reduce(out=t2_t[:], in_=red_view(m_t),
                                        axis=mybir.AxisListType.X,
                                        op=mybir.AluOpType.add)
                nc.vector.tensor_reduce(out=sv_t[:], in_=red_view(df_t),
                                        axis=mybir.AxisListType.X,
                                        op=mybir.AluOpType.add)

                def rf_col(c):
                    return bass.AP(rf_t.tensor, rf_t[:].offset + c,
                                   [rf_t[:].ap[0], (9, GRP), (3, 3)])

                def sv_col(c):
                    return bass.AP(sv_t.tensor, sv_t[:].offset + c,
                                   [sv_t[:].ap[0], (3, GRP), (0, 3)])

                # term1 = R_i @ sv, then add t2 and undo the 12-bit scale
                nc.vector.tensor_tensor(out=o2_t[:], in0=rf_col(0),
                                        in1=sv_col(0), op=mybir.AluOpType.mult)
                nc.vector.tensor_tensor(out=u2_t[:], in0=rf_col(1),
                                        in1=sv_col(1), op=mybir.AluOpType.mult)
                nc.vector.tensor_tensor(out=o2_t[:], in0=o2_t[:], in1=u2_t[:],
                                        op=mybir.AluOpType.add)
                nc.vector.tensor_tensor(out=u2_t[:], in0=rf_col(2),
                                        in1=sv_col(2), op=mybir.AluOpType.mult)
                nc.vector.tensor_tensor(out=o2_t[:], in0=o2_t[:], in1=u2_t[:],
                                        op=mybir.AluOpType.add)
                nc.vector.tensor_tensor(out=o2_t[:], in0=o2_t[:], in1=t2_t[:],
                                        op=mybir.AluOpType.add)
                nc.vector.tensor_scalar(
                    out=o16_t[:], in0=o2_t[:], scalar1=1.0 / SC,
                    scalar2=None, op0=mybir.AluOpType.mult)

                nc.sync.dma_start(
                    out=rhs[:, g * GRP * 3:(g + 1) * GRP * 3], in_=o16_t[:])
    nc.compile()
    return nc


def host_stage_core(c, xyz1, neighborList, weightMatrix, rotations):
    """Build core c's fused [BLOB_B] u8 section.  Per-core staging lets
    the caller overlap section c+1's CPU work with section c's (async)
    upload over the serialized axon link."""
    ngrp = NT // GRP
    ek = GRP * K
    lo = c * SLOTS
    hi = min((c + 1) * SLOTS, N_FULL)
    n = hi - lo
    blob = np.zeros(BLOB_B, dtype=np.uint8)
    table = blob[0:T_B].reshape(SLOTS, DB)
    p = np.ascontiguousarray(xyz1[0, lo:hi]).astype(np.float32, copy=False)
    table[:n, 0:6] = p.astype(np.float16).view(np.uint8)
    q = np.full((SLOTS, 9), 2048, dtype=np.uint16)
    r9 = rotations.reshape(N_FULL, 9)[lo:hi]
    np.add(np.clip(np.rint(r9 * SC), -2047, 2047), 2048,
           out=q[:n], casting="unsafe")
    for t in range(4):
        f0 = q[:, 2 * t]
        f1 = q[:, 2 * t + 1]
        table[:, 6 + 3 * t] = f0 & 255
        table[:, 7 + 3 * t] = (f0 >> 8) | ((f1 & 15) << 4)
        table[:, 8 + 3 * t] = f1 >> 4
    table[:, 18] = q[:, 8] & 255
    table[:, 19] = q[:, 8] >> 8
    nb = np.zeros((SLOTS, K), dtype=np.int32)
    nb[:n] = neighborList.reshape(N_FULL, K)[lo:hi]
    nb = nb.reshape(128, ngrp, ek)
    ob = blob[T_B:T_B + O_B].reshape(128, ngrp, 5 * ek // 2)
    np.bitwise_and(nb, 255, out=ob[:, :, 0:ek], casting="unsafe")
    np.bitwise_and(nb >> 8, 255, out=ob[:, :, ek:2 * ek], casting="unsafe")
    hi4 = (nb >> 16).astype(np.uint8)
    ob[:, :, 2 * ek:] = hi4[:, :, 0::2] | (hi4[:, :, 1::2] << 4)
    w = blob[T_B + O_B:].reshape(SLOTS, K)
    np.rint(weightMatrix.reshape(N_FULL, K)[lo:hi] * 255.0,
            casting="unsafe", out=w[:n])
    return blob


def _make_runner(nc):
    """Like bass2jax.run_bass_via_pjrt, but the jitted executable is built
    once and reused across kernel() calls (re-tracing + XLA re-compile per
    call costs ~1s against a ~1.5s invocation)."""
    import jax
    from jax.experimental.shard_map import shard_map
    from jax.sharding import Mesh, PartitionSpec
    from concourse import bass2jax

    bass2jax.install_neuronx_cc_hook()
    partition_name = (nc.partition_id_tensor.name
                      if nc.partition_id_tensor else None)
    in_names, out_names, out_avals = [], [], []
    for alloc in nc.m.functions[0].allocations:
        if not isinstance(alloc, mybir.MemoryLocationSet):
            continue
        name = alloc.memorylocations[0].name
        if alloc.kind == "ExternalInput":
            if name != partition_name:
                in_names.append(name)
        elif alloc.kind == "ExternalOutput":
            out_names.append(name)
            out_avals.append(jax.core.ShapedArray(
                tuple(alloc.tensor_shape), mybir.dt.np(alloc.dtype)))
    assert in_names == ["blob"] and out_names == ["rhs"], (in_names, out_names)
    n_params = len(in_names)
    n_outs = len(out_avals)
    all_names = in_names + out_names
    if partition_name is not None:
        all_names.append(partition_name)

    def _body(*args):
        operands = list(args)
        if partition_name is not None:
            operands.append(bass2jax.partition_id_tensor())
        outs = bass2jax._bass_exec_p.bind(
            *operands,
            out_avals=tuple(out_avals),
            in_names=tuple(all_names),
            out_names=tuple(out_names),
            lowering_input_output_aliases=(),
            sim_require_finite=True,
            sim_require_nnan=True,
            nc=nc,
        )
        return tuple(outs)

    devices = jax.devices()[:NCORES]
    mesh = Mesh(np.asarray(devices), ("core",))
    in_specs = (PartitionSpec("core"),) * (n_params + n_outs)
    out_specs = (PartitionSpec("core"),) * n_outs
    sharded = jax.jit(
        shard_map(_body, mesh=mesh, in_specs=in_specs, out_specs=out_specs,
                  check_rep=False),
        donate_argnums=tuple(range(n_params, n_params + n_outs)),
        keep_unused=True,
    )

    from jax.sharding import NamedSharding

    blob_sharding = NamedSharding(mesh, PartitionSpec("core"))

    def put_shard(section, c):
        # async transfer of one core's staged section
        return jax.device_put(section, devices[c])

    def run(shards):
        # assemble the already-(being-)uploaded per-core sections into
        # the global array the jitted executable expects
        blob_all = jax.make_array_from_single_device_arrays(
            (NCORES * BLOB_B,), blob_sharding, shards)
        # The kernel writes every rhs element, so the donated buffer's
        # contents are irrelevant.  Steady state donates the PREVIOUS
        # call's device-resident output array -- no 6 MB host upload for
        # the scratch buffer after the first call.
        buf = _CACHE.get("outbuf")
        if buf is None:
            buf = np.empty((NCORES * 128, NT * 3), np.float16)
        out, = sharded(blob_all, buf)
        _CACHE["outbuf"] = out
        return np.asarray(out)

    return put_shard, run


def kernel(xyz1, xyz2, neighborList, numNeighbors, accnumNeighbors,
           weightMatrix, rotations, arapWeight, trace=False):
    global LAST_EXEC_NS, LAST_RUN_WALL_S
    import time as _time
    xyz1 = np.asarray(xyz1)
    neighborList = np.asarray(neighborList)
    weightMatrix = np.asarray(weightMatrix)
    rotations = np.asarray(rotations)
    if "run" not in _CACHE:
        nc = build_kernel()
        _CACHE["put"], _CACHE["run"] = _make_runner(nc)
    # stage core c, then launch its async upload while staging core c+1:
    # the serialized axon link drains one section while the CPU packs the
    # next, hiding most of the staging time behind the transfer
    shards = []
    for c in range(NCORES):
        sec = host_stage_core(c, xyz1, neighborList, weightMatrix, rotations)
        shards.append(_CACHE["put"](sec, c))
    _t0 = _time.time()
    rhs_all = _CACHE["run"](shards)
    LAST_RUN_WALL_S = _time.time() - _t0
    rhs_all = rhs_all.reshape(NCORES, 128, NT * 3)
    parts = []
    for c in range(NCORES):
        lo = c * SLOTS
        hi = min((c + 1) * SLOTS, N_FULL)
        parts.append(rhs_all[c].reshape(SLOTS, 3)[:hi - lo])
    return np.concatenate(parts, axis=0).astype(np.float32)
